# revision 53
# baseline (speedup 1.0000x reference)
"""Trainium2 Bass kernel for the shifted-window attention block
(nn_Block_6373731467375), SPMD over 8 NeuronCores, data-parallel over batch.

Per core: 2 batch elements. Pass A computes the attention branch in rolled
window space (LN1 folded into qkv weights, dual-S softmax: S token-major for
denominators, S feature-major for the AV matmul), writes the scaled branch
output to a DRAM scratch in original token order. Pass B adds the residual,
applies LN2 (folded into fc1), runs the MLP and writes the final output.
"""

import numpy as np
import ml_dtypes

BF = ml_dtypes.bfloat16

DIM, H, HD, WS, SHIFT, NPATCH, MLP, EPS = 768, 12, 64, 128, 64, 128, 3072, 1e-5
B, N = 16, 2000
NCORES = 8
BL = B // NCORES          # batch elems per core
TOK = BL * N              # 4000
NW = 16                   # rolled 128-token tiles (=windows) per batch elem
NG = 4                    # groups of 4 tiles (512 tokens)
CC = DIM // 128           # 6 contraction chunks
JB = MLP // 128           # 24 hidden blocks
MAGIC = 0x5F3759DF

_CACHE = {}


# ---------------------------------------------------------------------------
# device kernel builder
# ---------------------------------------------------------------------------

def _fix_multi_waits(nc, mybir):
    """This walrus build rejects >1 sync-wait per instruction; hoist extra
    waits onto dedicated NOPs inserted just before, on the same engine."""
    n = 0
    for blk in nc.main_func.blocks:
        new_insts = []
        changed = False
        for ins in blk.instructions:
            si = ins.sync_info
            if si is not None and si.on_wait and len(si.on_wait) > 1:
                waits = list(si.on_wait)
                for w in waits[:-1]:
                    n += 1
                    nop = mybir.InstNoOp(
                        name=f"{ins.name}-sw{n}",
                        engine=ins.engine,
                        ins=[],
                        outs=[],
                        bass_nofuse=True,
                        sync_info=mybir.SyncInfo(on_wait=[w], on_update=[]),
                    )
                    new_insts.append(nop)
                si.on_wait = waits[-1:]
                changed = True
            new_insts.append(ins)
        if changed:
            blk.instructions = new_insts
    return n


CFG = {"swi": False}


def _build(fix_waits=True, passes=('A','B'), reps=1, gran=4, gp_x2=False,
           gp_emul=False, ops_pxb=False, qk_dve=3, pa_bufs=3, ps_bufs=2,
           swi=None, qk_f8=True, b_cop_dve=False):
    if swi is None:
        swi = CFG["swi"]
    import concourse.bass as bass
    import concourse.mybir as mybir
    from contextlib import ExitStack

    f32 = mybir.dt.float32
    bf16 = mybir.dt.bfloat16
    f8 = mybir.dt.float8e4
    u32 = mybir.dt.uint32
    DR = mybir.MatmulPerfMode.DoubleRow
    # weights-stationary MMs can use the software-interleaved layout
    DRW = (mybir.MatmulPerfMode.DoubleRowSwInterleave if swi else DR)
    AX = mybir.AxisListType
    OP = mybir.AluOpType
    AF = mybir.ActivationFunctionType

    from concourse.tile import TileContext

    nc = bass.Bass()
    p = {}
    p["xs"] = nc.declare_dram_parameter("xs", [TOK, DIM], f32, isOutput=False)
    p["xsb"] = nc.declare_dram_parameter("xsb", [TOK, DIM], bf16, isOutput=False)
    p["wqk"] = nc.declare_dram_parameter("wqk", [128, CC, 2 * DIM], f8, isOutput=False)
    p["wv"] = nc.declare_dram_parameter("wv", [128, CC, DIM], f8, isOutput=False)
    p["wproj"] = nc.declare_dram_parameter("wproj", [128, CC, DIM], f8, isOutput=False)
    p["wfc1"] = nc.declare_dram_parameter("wfc1", [128, CC, MLP], f8, isOutput=False)
    p["wfc2"] = nc.declare_dram_parameter("wfc2", [128, JB, DIM], f8, isOutput=False)
    p["bqk"] = nc.declare_dram_parameter("bqk", [128, 12], f32, isOutput=False)
    p["bfc1"] = nc.declare_dram_parameter("bfc1", [128, JB], f32, isOutput=False)
    p["bproj"] = nc.declare_dram_parameter("bproj", [128, DIM], f32, isOutput=False)
    p["bfc2x"] = nc.declare_dram_parameter("bfc2x", [128, DIM], f32, isOutput=False)
    p["bfc2r"] = nc.declare_dram_parameter("bfc2r", [1, DIM], bf16, isOutput=False)
    p["bprojr"] = nc.declare_dram_parameter("bprojr", [1, DIM], bf16, isOutput=False)
    p["bfeat"] = nc.declare_dram_parameter("bfeat", [128, 2, 6, WS], bf16, isOutput=False)
    p["bfeatm"] = nc.declare_dram_parameter("bfeatm", [128, 2, 6, WS], bf16, isOutput=False)
    p["ident"] = nc.declare_dram_parameter("ident", [128, 128], bf16, isOutput=False)
    p["ident8"] = nc.declare_dram_parameter("ident8", [128, 128], f8, isOutput=False)
    out_t = nc.declare_dram_parameter("out", [TOK, DIM], f32, isOutput=True)
    attn_scr = nc.dram_tensor("attn_scr", [TOK, DIM], bf16)

    with TileContext(nc) as tc, ExitStack() as ctx:
        cpool = ctx.enter_context(tc.tile_pool(name="consts", bufs=1))

        # resident constants
        sb = {}
        for name in ("bqk", "bfc1", "bproj",
                     "bfeat", "bfeatm", "ident", "ident8"):
            t = cpool.tile(list(p[name].shape), p[name].dtype, tag=name)
            nc.sync.dma_start(out=t[:], in_=p[name][:])
            sb[name] = t
        magic = cpool.tile([128, 1], u32, tag="magic")
        nc.vector.memset(magic[:], MAGIC)
        bfc2r = cpool.tile([1, DIM], bf16, tag="bfc2r")
        nc.sync.dma_start(out=bfc2r[:], in_=p["bfc2r"][:])
        bprojr = cpool.tile([1, DIM], bf16, tag="bprojr")
        nc.sync.dma_start(out=bprojr[:], in_=p["bprojr"][:])
        ones1 = cpool.tile([1, 128], bf16, tag="ones1")
        nc.vector.memset(ones1[:], 1.0)

        wB = ctx.enter_context(tc.tile_pool(name="wB", bufs=1))
        wfc1 = wB.tile([128, CC, MLP], f8)
        wfc2 = wB.tile([128, JB, DIM], f8)

        # ---------------- helpers ----------------
        def newton_rsqrt(pool, var_view, rstdg, tagp):
            """rstdg[:, :NG] = rsqrt(var_view + eps) via 3 fp32 Newton steps."""
            vts = pool.tile([128, NG], f32, tag=tagp + "v")
            y = pool.tile([128, NG], f32, tag=tagp + "y")
            t1 = pool.tile([128, NG], f32, tag=tagp + "t")
            nc.vector.tensor_scalar_add(out=vts[:], in0=var_view, scalar1=EPS)
            nc.vector.tensor_scalar(
                out=rstdg[:].bitcast(u32),
                in0=vts[:].bitcast(u32),
                scalar1=1,
                scalar2=None,
                op0=OP.logical_shift_right,
            )
            nc.vector.tensor_tensor(
                out=rstdg[:].bitcast(u32),
                in0=magic[:].to_broadcast([128, NG]),
                in1=rstdg[:].bitcast(u32),
                op=OP.subtract,
            )
            a, b = rstdg, y
            for _ in range(2):
                nc.vector.tensor_mul(out=t1[:], in0=a[:], in1=a[:])
                nc.vector.tensor_mul(out=t1[:], in0=t1[:], in1=vts[:])
                nc.vector.tensor_scalar(
                    out=t1[:], in0=t1[:], scalar1=-0.5, scalar2=1.5,
                    op0=OP.mult, op1=OP.add,
                )
                nc.vector.tensor_mul(out=b[:], in0=a[:], in1=t1[:])
                a, b = b, a
            assert a is rstdg  # odd iteration count lands in caller's tile

        def transpose6(pool, z_t, dst, tl, dt=bf16, cop_dve=False):
            """z_t [128,768] -> dst[:, :, tl*128:(tl+1)*128] ([128,6,128])."""
            zT = pool.tile([128, DIM], dt, tag="px")
            idt = sb["ident8"] if dt == f8 else sb["ident"]
            for cc in range(CC):
                nc.tensor.matmul(
                    zT[:, cc * 128 : (cc + 1) * 128],
                    z_t[:, cc * 128 : (cc + 1) * 128],
                    idt[:],
                    start=(cc == 0), stop=(cc == CC - 1),
                    is_transpose=True,
                )
            if cop_dve:
                nc.vector.tensor_copy(
                    out=dst[:, :, tl * 128 : (tl + 1) * 128],
                    in_=zT[:].rearrange("p (c q) -> p c q", c=CC),
                )
            else:
                nc.scalar.activation(
                    out=dst[:, :, tl * 128 : (tl + 1) * 128],
                    in_=zT[:].rearrange("p (c q) -> p c q", c=CC),
                    func=AF.Copy,
                )

        # =================== PASSES (A/B interleaved via deps) ==========
        if True:
         with tc.tile_pool(name="wA", bufs=1) as wA, \
             tc.tile_pool(name="pa", bufs=pa_bufs) as pa, \
             tc.tile_pool(name="pa1", bufs=1) as pa1, \
             tc.tile_pool(name="pa3", bufs=2) as pa3, \
             tc.tile_pool(name="pb", bufs=2) as pb, \
             tc.tile_pool(name="pb3", bufs=2) as pb3, \
             tc.tile_pool(name="pg", bufs=1) as pg, \
             tc.tile_pool(name="pxA", bufs=2, space="PSUM") as pxA, \
             tc.tile_pool(name="pxB", bufs=1, space="PSUM") as pxB, \
             tc.tile_pool(name="ps", bufs=ps_bufs, space="PSUM") as ps:

            wqk = wA.tile([128, CC, 2 * DIM], f8)
            wv = wA.tile([128, CC, DIM], f8)
            wproj = wA.tile([128, CC, DIM], f8)

            def emit_A_head(b, g, rep=0):
                x0 = b * N
                if True:
                    if rep == 0 and b == 0 and g == 1:
                        # prefetch MLP weights once the startup DMA burst clears
                        nc.gpsimd.dma_start(out=wfc1[:], in_=p["wfc1"][:])
                        nc.gpsimd.dma_start(out=wfc2[:], in_=p["wfc2"][:])
                    hT = pa.tile([128, CC, 512], f8, tag="hT")
                    first = rep == 0 and b == 0 and g == 0
                    mvg = pa.tile([128, NG, 2], f32, tag="mvg")
                    rstdg = pa.tile([128, NG], f32, tag="rstdg")
                    xG = pa.tile([128, NG, DIM], bf16, tag="xG")
                    for tl in range(4):
                        t = 4 * g + tl
                        if t < NW - 1:
                            nc.sync.dma_start(
                                out=xG[:, tl, :],
                                in_=p["xsb"][x0 + 64 + 128 * t : x0 + 64 + 128 * (t + 1)],
                            )
                        else:
                            nc.vector.memset(xG[:, tl, :], 0.0)
                            nc.sync.dma_start(
                                out=xG[0:16, tl, :], in_=p["xsb"][x0 + 1984 : x0 + 2000]
                            )
                            nc.sync.dma_start(
                                out=xG[64:128, tl, :], in_=p["xsb"][x0 : x0 + 64]
                            )
                        stats = pa3.tile([128, 2, 6], f32, tag="ln_stats")
                        nc.vector.bn_stats(out=stats[:, 0, :], in_=xG[:, tl, 0:512])
                        nc.vector.bn_stats(out=stats[:, 1, :], in_=xG[:, tl, 512:768])
                        nc.vector.bn_aggr(out=mvg[:, tl, :], in_=stats[:])
                    if first:
                        # weights follow the first group's x tiles on the queues
                        for fblk in range(4):
                            nc.sync.dma_start(
                                out=wqk[:, :, fblk * 384 : (fblk + 1) * 384],
                                in_=p["wqk"][:, :, fblk * 384 : (fblk + 1) * 384],
                            )
                        nc.sync.dma_start(out=wv[:], in_=p["wv"][:])
                        nc.sync.dma_start(out=wproj[:], in_=p["wproj"][:])
                    newton_rsqrt(pa3, mvg[:, :, 1], rstdg, "nra")
                    for tl in range(4):
                        z_t = pa3.tile([128, DIM], bf16, tag="z_t")
                        nc.vector.tensor_scalar(
                            out=z_t[:], in0=xG[:, tl, :],
                            scalar1=mvg[:, tl, 0:1], scalar2=rstdg[:, tl : tl + 1],
                            op0=OP.subtract, op1=OP.mult,
                        )
                        transpose6(pxA, z_t, hT, tl)

                    # qkv for the group
                    QKT = pa.tile([128, 12, 512], f8 if qk_f8 else bf16,
                                  tag="QKT")
                    for fb in range(12):
                        qk_ps = pxA.tile([128, 512], f32, tag="px")
                        for cc2 in range(CC // 2):
                            nc.tensor.matmul(
                                qk_ps[:],
                                wqk[:, 2 * cc2 : 2 * cc2 + 2, fb * 128 : (fb + 1) * 128],
                                hT[:, 2 * cc2 : 2 * cc2 + 2, :],
                                start=(cc2 == 0), stop=(cc2 == CC // 2 - 1),
                                perf_mode=DRW,
                            )
                        if fb >= 12 - qk_dve:
                            # K blocks (scale=1): evac on DVE to offload ACT
                            nc.vector.tensor_scalar(
                                out=QKT[:, fb, :], in0=qk_ps[:],
                                scalar1=sb["bqk"][:, fb : fb + 1], scalar2=None,
                                op0=OP.add,
                            )
                        else:
                            nc.scalar.activation(
                                out=QKT[:, fb, :], in_=qk_ps[:],
                                func=AF.Identity,
                                bias=sb["bqk"][:, fb : fb + 1],
                                scale=(0.125 if fb < 6 else 1.0),
                            )
                    # V with a ones column per head: AV emits softmax
                    # denominators (col 64) alongside the head outputs.
                    VG = pa1.tile([128, 4, H, HD + 1], bf16, tag="VG")
                    nc.vector.memset(VG[:, :, :, HD : HD + 1], 1.0)
                    for tl in range(4):
                        v_ps = pxB.tile([128, 2, 512], f32, tag="pxb")
                        for cc2 in range(CC // 2):
                            for nh in range(2):
                                nc.tensor.matmul(
                                    v_ps[:, nh, 0:384],
                                    hT[:, 2 * cc2 : 2 * cc2 + 2, tl * 128 : (tl + 1) * 128],
                                    wv[:, 2 * cc2 : 2 * cc2 + 2, nh * 384 : (nh + 1) * 384],
                                    start=(cc2 == 0), stop=(cc2 == CC // 2 - 1),
                                    perf_mode=DR,
                                )
                        nc.scalar.activation(
                            out=VG[:, tl, :, 0:HD].rearrange(
                                "p (a h) e -> p a h e", a=2
                            ),
                            in_=v_ps[:, :, 0:384].rearrange(
                                "p a (h e) -> p a h e", h=6
                            ),
                            func=AF.Copy,
                        )

                    return QKT, VG

            def emit_A_wins(b, g, QKT, VG):
                x0 = b * N
                if True:
                    for tl in range(4):
                        t = 4 * g + tl
                        masked = t == NW - 1
                        bfeat_t = sb["bfeatm"] if masked else sb["bfeat"]
                        qs = slice(tl * 128, (tl + 1) * 128)

                        # Even/odd heads target different PSUM banks: MMs with
                        # disjoint PE row-groups (base partition 0 vs 64) run
                        # concurrently, and concurrent writes to one PSUM bank
                        # hard-fault the device. Slot j: even i -> i//2 (bank
                        # 0), odd i -> 4 + i//2 (bank 1).
                        e_feat = []
                        for half in range(2):
                            hh = list(range(half * 6, half * 6 + 6))

                            def _v(t):  # [128,8,128] -> [128,2,3,128] skipping slots 3,7
                                return t[:].rearrange(
                                    "p (g j) k -> p g j k", g=2
                                )[:, :, 0:3, :]

                            s_feat = ps.tile([128, 8, 128], f32, tag="s")
                            for i, h in enumerate(hh):
                                bp = (h % 2) * 64
                                j = (i // 2) + 4 * (i % 2)
                                nc.tensor.matmul(
                                    s_feat[:, j, :],
                                    QKT[bp : bp + 64, 6 + h // 2, qs],
                                    QKT[bp : bp + 64, h // 2, qs],
                                    start=(i in (0, 1)), stop=(i in (4, 5)),
                                )
                            E_f = pa.tile([128, 8, 128], bf16, tag="E_feat")
                            nc.scalar.activation(
                                out=_v(E_f), in_=_v(s_feat), func=AF.Exp
                            )
                            eng_em = nc.gpsimd if gp_emul else nc.vector
                            eng_em.tensor_mul(
                                out=_v(E_f), in0=_v(E_f),
                                in1=bfeat_t[:, half, :, :].rearrange(
                                    "p (g j) k -> p g j k", g=2
                                ),
                            )
                            e_feat.append(E_f)

                        # AV with ones column: O_ps[:, g, hh*65+64] = denom
                        O_ps = (pxB if ops_pxb else ps).tile(
                            [128, 2, 512], f32, tag="pxb" if ops_pxb else "s")
                        for h in range(H):
                            i = h % 6
                            j = (i // 2) + 4 * (i % 2)
                            nc.tensor.matmul(
                                O_ps[:, h // 6, (h % 6) * 65 : (h % 6) * 65 + 65],
                                e_feat[h // 6][:, j, :],
                                VG[:, tl, h, :],
                                start=(h in (0, 6)), stop=(h in (5, 11)),
                            )
                        rden = pa.tile([128, 2, 6, 1], f32, tag="rden")
                        nc.vector.reciprocal(
                            out=rden[:],
                            in_=O_ps[:, :, 0:390].rearrange(
                                "p g (h e) -> p g h e", e=65
                            )[:, :, :, 64:65],
                        )
                        Osb = pa.tile([128, DIM], bf16, tag="Osb")
                        nc.vector.tensor_tensor(
                            out=Osb[:].rearrange("p (a h e) -> p a h e", a=2, h=6),
                            in0=O_ps[:, :, 0:390].rearrange(
                                "p a (h e) -> p a h e", e=65
                            )[:, :, :, 0:64],
                            in1=rden[:].to_broadcast([128, 2, 6, 64]),
                            op=OP.mult,
                        )
                        OTsb = pa.tile([128, CC, 128], f8, tag="OTsb")
                        OT_ps = pxA.tile([128, DIM], bf16, tag="px")
                        for cc in range(CC):
                            nc.tensor.transpose(
                                out=OT_ps[:, cc * 128 : (cc + 1) * 128],
                                in_=Osb[:, cc * 128 : (cc + 1) * 128],
                                identity=sb["ident"][:],
                            )
                        nc.scalar.activation(
                            out=OTsb[:],
                            in_=OT_ps[:].rearrange("p (c q) -> p c q", c=CC),
                            func=AF.Copy,
                        )
                        pr_ps = pxB.tile([128, 2, 512], f32, tag="pxb")
                        for cc2 in range(CC // 2):
                            for nh in range(2):
                                nc.tensor.matmul(
                                    pr_ps[:, nh, 0:384],
                                    OTsb[:, 2 * cc2 : 2 * cc2 + 2, :],
                                    wproj[:, 2 * cc2 : 2 * cc2 + 2, nh * 384 : (nh + 1) * 384],
                                    start=(cc2 == 0), stop=False,
                                    perf_mode=DR,
                                )
                        for nh in range(2):
                            nc.tensor.matmul(
                                pr_ps[:, nh, 0:384],
                                ones1[:],
                                bprojr[:, nh * 384 : (nh + 1) * 384],
                                start=False, stop=True,
                            )
                        att = pa1.tile([128, DIM], bf16, tag="att")
                        nc.scalar.activation(
                            out=att[:].rearrange("p (a n) -> p a n", a=2),
                            in_=pr_ps[:, :, 0:384],
                            func=AF.Copy,
                        )
                        if t < NW - 1:
                            nc.sync.dma_start(
                                out=attn_scr[x0 + 64 + 128 * t : x0 + 64 + 128 * (t + 1)],
                                in_=att[:],
                            )
                        else:
                            nc.sync.dma_start(
                                out=attn_scr[x0 + 1984 : x0 + 2000], in_=att[0:16, :]
                            )
                            nc.sync.dma_start(out=attn_scr[x0 : x0 + 64], in_=att[64:128, :])

            def emit_B_head(b, g):
                x0 = b * N
                if True:
                    hT = pb.tile([128, CC, 512], f8, tag="hT2")
                    mvg = pb.tile([128, NG, 2], f32, tag="mvg2")
                    rstdg = pb.tile([128, NG], f32, tag="rstdg2")
                    x2G = pb.tile([128, NG, DIM], f32, tag="x2G")
                    tss = []
                    for tl in range(4):
                        m = 4 * g + tl
                        ts = min(128, N - 128 * m)
                        tss.append(ts)
                        x_m = pb3.tile([128, DIM], f32, tag="x_m")
                        a_m = pb3.tile([128, DIM], bf16, tag="a_m")
                        nc.sync.dma_start(
                            out=x_m[:ts], in_=p["xs"][x0 + 128 * m : x0 + 128 * m + ts]
                        )
                        nc.sync.dma_start(
                            out=a_m[:ts], in_=attn_scr[x0 + 128 * m : x0 + 128 * m + ts]
                        )
                        if ts < 128:
                            nc.vector.memset(x2G[:, tl, :], 0.0)
                        eng_x2 = nc.gpsimd if gp_x2 else nc.vector
                        eng_x2.tensor_add(
                            out=x2G[:ts, tl, :], in0=x_m[:ts], in1=a_m[:ts]
                        )
                        stats = pb3.tile([128, 2, 6], f32, tag="ln_stats2")
                        nc.vector.bn_stats(out=stats[:ts, 0, :], in_=x2G[:ts, tl, 0:512])
                        nc.vector.bn_stats(out=stats[:ts, 1, :], in_=x2G[:ts, tl, 512:768])
                        if ts < 128:
                            nc.vector.memset(mvg[:, tl, :], 0.0)
                        nc.vector.bn_aggr(out=mvg[:ts, tl, :], in_=stats[:ts])
                    newton_rsqrt(pb3, mvg[:, :, 1], rstdg, "nrb")
                    for tl in range(4):
                        ts = tss[tl]
                        z2 = pb3.tile([128, DIM], bf16, tag="z2")
                        if ts < 128:
                            nc.vector.memset(z2[:], 0.0)
                        nc.vector.tensor_scalar(
                            out=z2[:ts], in0=x2G[:ts, tl, :],
                            scalar1=mvg[:ts, tl, 0:1], scalar2=rstdg[:ts, tl : tl + 1],
                            op0=OP.subtract, op1=OP.mult,
                        )
                        transpose6(pxA, z2, hT, tl, cop_dve=b_cop_dve)

                    gT = pg.tile([128, JB, 512], f8, tag="gT")
                    for jb in range(JB):
                        f_ps = pxA.tile([128, 512], f32, tag="px")
                        for cc2 in range(CC // 2):
                            nc.tensor.matmul(
                                f_ps[:],
                                wfc1[:, 2 * cc2 : 2 * cc2 + 2, jb * 128 : (jb + 1) * 128],
                                hT[:, 2 * cc2 : 2 * cc2 + 2, :],
                                start=(cc2 == 0), stop=(cc2 == CC // 2 - 1),
                                perf_mode=DRW,
                            )
                        nc.scalar.activation(
                            out=gT[:, jb, :], in_=f_ps[:], func=AF.Gelu,
                            bias=sb["bfc1"][:, jb : jb + 1], scale=1.0,
                        )
                    return gT, x2G, tss

            def emit_B_tail(b, g, gT, x2G, tss):
                x0 = b * N
                if True:
                    for tl in range(4):
                        m = 4 * g + tl
                        ts = tss[tl]
                        m_ps = pxB.tile([128, 2, 512], f32, tag="pxb")
                        for hc2 in range(JB // 2):
                            for nh in range(2):
                                nc.tensor.matmul(
                                    m_ps[:, nh, 0:384],
                                    gT[:, 2 * hc2 : 2 * hc2 + 2, tl * 128 : (tl + 1) * 128],
                                    wfc2[:, 2 * hc2 : 2 * hc2 + 2, nh * 384 : (nh + 1) * 384],
                                    start=(hc2 == 0), stop=False,
                                    perf_mode=DR,
                                )
                        # fc2 bias via a rank-1 accumulating matmul (frees DVE)
                        for nh in range(2):
                            nc.tensor.matmul(
                                m_ps[:, nh, 0:384],
                                ones1[:],
                                bfc2r[:, nh * 384 : (nh + 1) * 384],
                                start=False, stop=True,
                            )
                        o_sb = pb3.tile([128, DIM], f32, tag="o_sb")
                        nc.vector.tensor_tensor(
                            out=o_sb[:ts].rearrange("p (a n) -> p a n", a=2),
                            in0=m_ps[:ts, :, 0:384],
                            in1=x2G[:ts, tl, :].rearrange("p (a n) -> p a n", a=2),
                            op=OP.add,
                        )
                        nc.sync.dma_start(
                            out=out_t[x0 + 128 * m : x0 + 128 * m + ts], in_=o_sb[:ts]
                        )

            def emit_B_block(pairs):
                # one-ahead head/tail pipelining measured slower on HW;
                # emit each group's head and tail together
                for (b, g) in pairs:
                    st = emit_B_head(b, g)
                    emit_B_tail(b, g, *st)

            def emit_B(b, g):
                emit_B_block([(b, g)])

            def emit_A_block(pairs):
                for (b, g, rep) in pairs:
                    st = emit_A_head(b, g, rep)
                    emit_A_wins(b, g, *st)

            def emit_A(b, g, rep=0):
                emit_A_block([(b, g, rep)])

            for rep in range(reps):
                if gran == "ab":
                    # phase-separated, batch-paired: two independent A
                    # streams interleave, then two B streams
                    for g in range(NG):
                        emit_A(0, g, rep)
                        emit_A(1, g, rep)
                    for gB in (1, 2, 3, 0):
                        emit_B(0, gB)
                        emit_B(1, gB)
                    continue
                if gran == "fine":
                    # pipeline B(b,g) right behind its A deps, across both
                    # batch elems: head = 2 A-groups, tail = 1 B-group
                    emit_A(0, 0, rep)
                    emit_A(0, 1, rep)
                    emit_B(0, 1)
                    emit_A(0, 2, rep)
                    emit_B(0, 2)
                    emit_A(0, 3, rep)
                    emit_B(0, 3)
                    emit_A(1, 0, rep)
                    emit_B(0, 0)
                    emit_A(1, 1, rep)
                    emit_B(1, 1)
                    emit_A(1, 2, rep)
                    emit_B(1, 2)
                    emit_A(1, 3, rep)
                    emit_B(1, 3)
                    emit_B(1, 0)
                    continue
                if gran in (4, "p"):
                    emit_A_block([(0, g, rep) for g in range(NG)])
                else:
                    for g in range(NG):
                        emit_A(0, g, rep)
                if gran == 1:
                    for gB, gA in zip((1, 2, 3, 0), range(NG)):
                        emit_B(0, gB)
                        emit_A(1, gA, rep)
                elif gran == 2:
                    for gBs, gAs in (((1, 2), (0, 1)), ((3, 0), (2, 3))):
                        for gB in gBs:
                            emit_B(0, gB)
                        for gA in gAs:
                            emit_A(1, gA, rep)
                else:  # gran == 4
                    emit_B_block([(0, gB) for gB in (1, 2, 3, 0)])
                    emit_A_block([(1, gA, rep) for gA in range(NG)])
                emit_B_block([(1, gB) for gB in (1, 2, 3, 0)])

    if fix_waits:
        nsplit = _fix_multi_waits(nc, mybir)
        print(f"_fix_multi_waits: split {nsplit} waits", flush=True)
    return nc


# ---------------------------------------------------------------------------
# host preprocessing
# ---------------------------------------------------------------------------

def _bf(x):
    return np.ascontiguousarray(np.asarray(x, np.float32).astype(BF))


F8 = ml_dtypes.float8_e4m3


def _f8(x):
    return np.ascontiguousarray(np.asarray(x, np.float32).astype(F8))


def _swi_weights(w):
    """w [128, CC, OUT]; MM lhsT blocks are [:, 2c:2c+2, b*128:(b+1)*128].
    Rearrange for DoubleRowSwInterleave (A/B pairs interleaved, cols
    reversed) so the logical matmul is unchanged."""
    p, cc, out = w.shape
    v = w.reshape(p, cc // 2, 2, out // 128, 128)
    s = np.arange(2)[:, None] * 128 + np.arange(128)[None, :]
    jj = s % 2
    tt = 127 - s // 2
    r = v[:, :, jj, :, tt]
    st = r.transpose(2, 3, 0, 4, 1)
    return np.ascontiguousarray(st.reshape(p, cc, out))


def _precompute(inp):
    qkv_w = np.asarray(inp["qkv_w"], np.float32)
    qkv_b = np.asarray(inp["qkv_b"], np.float32)
    n1w, n1b = np.asarray(inp["norm1_w"], np.float32), np.asarray(inp["norm1_b"], np.float32)
    n2w, n2b = np.asarray(inp["norm2_w"], np.float32), np.asarray(inp["norm2_b"], np.float32)
    proj_w, proj_b = np.asarray(inp["proj_w"], np.float32), np.asarray(inp["proj_b"], np.float32)
    ls1, ls2 = np.asarray(inp["ls1"], np.float32), np.asarray(inp["ls2"], np.float32)
    fc1_w, fc1_b = np.asarray(inp["fc1_w"], np.float32), np.asarray(inp["fc1_b"], np.float32)
    fc2_w, fc2_b = np.asarray(inp["fc2_w"], np.float32), np.asarray(inp["fc2_b"], np.float32)
    rel_bias = np.asarray(inp["rel_bias"], np.float32)

    c = {}
    swi = CFG["swi"]
    wqk = _f8(n1w[:, None] * qkv_w[:, : 2 * DIM])           # [768, 1536]
    c["wqk"] = np.ascontiguousarray(wqk.reshape(CC, 128, 2 * DIM).transpose(1, 0, 2))
    if swi:
        c["wqk"] = _swi_weights(c["wqk"])
    wv = _f8(n1w[:, None] * qkv_w[:, 2 * DIM :])
    c["wv"] = np.ascontiguousarray(wv.reshape(CC, 128, DIM).transpose(1, 0, 2))
    qkvb_f = n1b @ qkv_w + qkv_b
    bqk = qkvb_f[: 2 * DIM].reshape(12, 128).T.astype(np.float32).copy()
    bqk[:, :6] *= 0.125
    c["bqk"] = np.ascontiguousarray(bqk)
    bv = qkvb_f[2 * DIM :]
    wproj = _f8(proj_w * ls1[None, :])
    c["wproj"] = np.ascontiguousarray(wproj.reshape(CC, 128, DIM).transpose(1, 0, 2))
    bproj_row = ((bv @ proj_w + proj_b) * ls1).astype(np.float32)
    c["bproj"] = np.ascontiguousarray(np.broadcast_to(bproj_row, (128, DIM)))
    c["bprojr"] = _bf(bproj_row.reshape(1, DIM))
    wfc1 = _f8(n2w[:, None] * fc1_w)
    c["wfc1"] = np.ascontiguousarray(wfc1.reshape(CC, 128, MLP).transpose(1, 0, 2))
    if swi:
        c["wfc1"] = _swi_weights(c["wfc1"])
    c["bfc1"] = np.ascontiguousarray(
        (n2b @ fc1_w + fc1_b).reshape(JB, 128).T.astype(np.float32)
    )
    wfc2 = _f8(fc2_w * ls2[None, :])
    c["wfc2"] = np.ascontiguousarray(wfc2.reshape(JB, 128, DIM).transpose(1, 0, 2))
    c["bfc2x"] = np.ascontiguousarray(
        np.broadcast_to((fc2_b * ls2).astype(np.float32), (128, DIM))
    )
    c["bfc2r"] = _bf((fc2_b * ls2).reshape(1, DIM))

    coords = np.arange(WS)
    rel_idx = coords[None, :] - coords[:, None] + (NPATCH - 1)
    Bmat = rel_bias[rel_idx].transpose(2, 0, 1).astype(np.float32)  # [H, q, k]
    maskrow = np.zeros(WS, np.float32)
    maskrow[16:64] = -30000.0
    Bm = Bmat + maskrow[None, None, :]
    # head order per half: evens then odds (matches S-slot blocks)
    horder = [0, 2, 4, 1, 3, 5]

    def _blocked(mat):  # mat [H, a, b] -> [a, 2, 6, b] exp'd, bf16
        e = np.exp(mat)
        out = np.stack(
            [np.stack([e[6 * half + i] for i in horder], 0) for half in range(2)], 0
        )  # [2, 6, a, b]
        return _bf(out.transpose(2, 0, 1, 3))

    c["bfeat"] = _blocked(Bmat.transpose(0, 2, 1))
    c["bfeatm"] = _blocked(Bm.transpose(0, 2, 1))
    c["ident"] = _bf(np.eye(128, dtype=np.float32))
    c["ident8"] = _f8(np.eye(128, dtype=np.float32))
    return c


def kernel(**inputs):
    from concourse.bass_utils import run_bass_kernel_spmd

    if "nc" not in _CACHE:
        _CACHE["nc"] = _build()
    nc = _CACHE["nc"]

    c = _precompute(inputs)
    x = np.asarray(inputs["x"], np.float32)  # [16, 2000, 768]
    in_maps = []
    for core in range(NCORES):
        m = dict(c)
        m["xs"] = np.ascontiguousarray(
            x[core * BL : (core + 1) * BL].reshape(TOK, DIM)
        )
        m["xsb"] = m["xs"].astype(BF)
        in_maps.append(m)
    res = run_bass_kernel_spmd(nc, in_maps, core_ids=list(range(NCORES)))
    out = np.stack(
        [res.results[i]["out"].reshape(BL, N, DIM) for i in range(NCORES)]
    ).reshape(B, N, DIM)
    return out.astype(np.float32)

